# revision 3
# baseline (speedup 1.0000x reference)
"""CARAFE forward on 8 TRN2 NeuronCores.

Problem: features (8,128,64,64) f32, masks (8,25,128,128) f32
         -> out (8,128,128,128) f32, KERNEL_SIZE=5, GROUP=1, SCALE=2.

Sharding: pure data-parallel, one batch sample per core.

Per-core formulation (banded matmul):
  out[c, 2h+p, 2w+q] = sum_{i,j} f[c, h+i-2, w+j-2] * m[i*5+j, 2h+p, 2w+q]
For each (h, i) this is a matmul over x = w+j-2 (K=64, W axis):
  PSUM[c, col(p,w,q)] += sum_x FT[x, r, c] * S[x, h, i, col]
with FT[x, h, c] = f[c, h, x] and the banded mask matrix
  S[w+j-2, h, i, p*128 + 2w+q] = m[5i+j, 2h+p, 2w+q]   (zeros elsewhere;
band entries falling outside x in [0,64) would multiply zero-padded feature
columns, so they are simply dropped). Both FT and S are prepared host-side
(pure layout; all FLOPs happen on device). Matmuls accumulate fp32 in PSUM
over the valid i's, DVE-evacuate to SBUF, DMA to the DRAM output.

VARIANT (env CARAFE_VARIANT): "fp16" (default) ships FT/S as float16
(~4e-4 rel err, half the DMA traffic); "f32r" ships float32 and runs the PE
in float32r (TF32-like, ~1.4e-4 rel err).
"""

import os

import numpy as np

VARIANT = os.environ.get("CARAFE_VARIANT", "fp16")

N_CORES = 8
C, H, W = 128, 64, 64
K5 = 5
PAD = 2
KX = W                    # contraction length (padded-out rows dropped)
NCOL = 256                # (p, wo) = 2 * 128 output columns per low-res row h
H_CHUNK = 4               # h rows per S-stream chunk

_compiled = None


def _dts():
    import concourse.mybir as mybir

    if VARIANT == "fp16":
        return mybir.dt.float16, np.float16
    return mybir.dt.float32r, np.float32


def _build_program(n_iters: int = 1):
    """Build the SPMD bass program. n_iters > 1 wraps the body in a hardware
    loop for timing (tunnel overhead cancels in wall-clock deltas)."""
    import concourse.bacc as bacc
    import concourse.mybir as mybir
    import concourse.tile as tile

    in_dt, _ = _dts()
    nc = bacc.Bacc("TRN2", target_bir_lowering=False, debug=False,
                   num_devices=N_CORES)

    ft = nc.dram_tensor("ft", [KX, H, C], in_dt, kind="ExternalInput")
    s = nc.dram_tensor("s", [KX, H, K5, NCOL], in_dt, kind="ExternalInput")
    out = nc.dram_tensor("out", [C, 2 * H, 2 * W], mybir.dt.float32,
                         kind="ExternalOutput")

    def body(tc, sb, ps, ob, ss):
        ft_t = sb.tile([KX, H, C], in_dt, tag="ft")
        nc.sync.dma_start(ft_t[:], ft[:])
        for h0 in range(0, H, H_CHUNK):
            s_t = ss.tile([KX, H_CHUNK, K5, NCOL], in_dt, tag="s")
            nc.sync.dma_start(s_t[:], s[:, h0:h0 + H_CHUNK, :, :])
            for hl in range(H_CHUNK):
                h = h0 + hl
                iv = [i for i in range(K5) if 0 <= h + i - PAD < H]
                acc = ps.tile([C, NCOL], mybir.dt.float32)
                for n_i, i in enumerate(iv):
                    r = h + i - PAD
                    nc.tensor.matmul(acc[:], ft_t[:, r, :], s_t[:, hl, i, :],
                                     start=(n_i == 0), stop=(n_i == len(iv) - 1))
                o = ob.tile([C, NCOL], mybir.dt.float32, tag="o")
                nc.vector.tensor_copy(o[:], acc[:])
                nc.sync.dma_start(
                    out[:, 2 * h:2 * h + 2, :],
                    o[:].rearrange("c (p w) -> c p w", p=2))

    with tile.TileContext(nc) as tc:
        with (
            tc.tile_pool(name="sb", bufs=1) as sb,
            tc.tile_pool(name="ss", bufs=3) as ss,
            tc.tile_pool(name="ps", bufs=8, space="PSUM") as ps,
            tc.tile_pool(name="ob", bufs=4) as ob,
        ):
            if n_iters == 1:
                body(tc, sb, ps, ob, ss)
            else:
                with tc.For_i(0, n_iters, 1):
                    body(tc, sb, ps, ob, ss)

    nc.compile()
    return nc


def _prep_inputs(features: np.ndarray, masks: np.ndarray):
    """Host-side layout prep (no FLOPs): per-sample FT and banded S."""
    _, np_dt = _dts()
    n = features.shape[0]
    ft = np.ascontiguousarray(features.transpose(0, 3, 2, 1)).astype(np_dt)

    # masks [n, 25, 128, 128] -> m[n, i, j, h, p, w, q]
    m = masks.reshape(n, K5, K5, H, 2, W, 2)
    s = np.zeros((n, KX, H, K5, 2, W, 2), dtype=np_dt)
    for j in range(K5):
        # x = w + j - PAD must be in [0, KX); clip the w range accordingly
        wlo = max(0, PAD - j)
        whi = min(W, W + PAD - j)
        wi = np.arange(wlo, whi)
        # m[:, :, j] dims (n, i, h, p, w, q) -> (w, n, h, i, p, q)
        s[:, wi + j - PAD, :, :, :, wi, :] = (
            m[:, :, j, :, :, wlo:whi].transpose(4, 0, 2, 1, 3, 5).astype(np_dt))
    s = s.reshape(n, KX, H, K5, NCOL)
    return ft, s


def kernel(features: np.ndarray, masks: np.ndarray) -> np.ndarray:
    from concourse.bass_utils import run_bass_kernel_spmd

    global _compiled
    if _compiled is None:
        _compiled = _build_program(1)
    nc = _compiled

    ft, s = _prep_inputs(np.asarray(features, dtype=np.float32),
                         np.asarray(masks, dtype=np.float32))
    in_maps = [{"ft": ft[i], "s": s[i]} for i in range(N_CORES)]
    res = run_bass_kernel_spmd(nc, in_maps, list(range(N_CORES)))
    return np.stack([res.results[i]["out"] for i in range(N_CORES)], axis=0)
